# revision 30
# baseline (speedup 1.0000x reference)
"""Multi-head attention Bass kernel v7 for Trainium2, SPMD over 8 NeuronCores.

Problem: q,k,v [4, 16, 2048, 64] fp32 -> softmax(q@k^T/sqrt(64))@v.
64 (batch*head) heads, 8 consecutive heads per core, no cross-core
communication.

Baseline-v2 scheduler (strict kt-ordered PV ratchet, tuned lags) with three
orthogonal upgrades:

  1. HOST-PACKED INPUTS.  The host pre-casts to f16 (q,k scaled 2^-4 each)
     and pre-packs the transposed layouts (qt/kt [128,1024]: partitions
     0:64 = d of even n, 64:128 = d of odd n; kw = kt with partition halves
     swapped; vt [128,16,64] for the vaug slots).  No device xbar-transpose,
     no on-device k-swap copies; the first head's loads are chunked and
     spread over the SP/ACT HWDGE queues + gpsimd SWDGE so the first S
     matmul is gated only by a 128-col k chunk and a 512-col q half.
  2. HOST-SIDE NORMALIZATION.  The O psum banks hold 4 q-tiles x (64 cols +
     ones-column denominator).  The drain copies psum->sbuf and DMAs the
     raw [128, 260] block; the host does num/denom.  Drops the gpsimd
     normalize pass and the ost staging entirely.
  3. POOL-ASSISTED EXP CHAINS.  Per parity, k-tiles 1 and 7 compute
     exp via cubic(z/16)^4 on DVE (1 op), one builtin DVE f16 mul (2x mode)
     -> q^8, one GPSIMD mul -> q^16.  k-tiles 4, 10, 13 use the 2-op
     z/32 DVE path; everything else stays on ACT.  This moves ~2 of ACT's
     ~12 per-parity blocks to the (otherwise idle) Pool engine.

Per-core per-head algorithm otherwise identical to baseline v2: 32 S-steps
(q-parity x 16 k-tiles), S^T [128,1024] f32 psum ring of 3, PV natural
orientation accumulating [128, 65] O blocks over 16 k-tiles.
PSUM: 3x[128,1024] S ring (6 banks) + 2 O banks.
"""

import numpy as np

B, H, N, D = 4, 16, 2048, 64
NCORES = 8
HEADS = B * H          # 64
HPC = HEADS // NCORES  # 8 heads per core
NT = 16                # k tiles of 128 rows (0-7 even k, 8-15 odd k)
NSTEP = 32             # steps per head = 2 parities x 16 k-tiles
ACT_SCALE = 32.0

_CACHE = {}

# z/32 scheme (DVE 2-op): exp(32 s) = q(s)^32, cubic minimax on |s|<=7.2/32.
EXP_C1 = 1.0000400173833472
EXP_C2 = 0.5014175146307196
EXP_C3 = 0.16555244796209398
# z/16 scheme (pool chain): exp(32 s) = q(s)^16 with q ~ e^{2s}; coefficients
# are the |v|<=0.45 minimax cubic with v=2s folded in (x2, x4, x8).
Z16_C1 = 2.0011006099212336
Z16_C2 = 2.0277839111113912
Z16_C3 = 1.3135423890325262

# k-tiles (per parity) whose exp runs on the 2-op z/32 DVE path.
OFFKT_P = {0: (1, 4, 7, 10, 13), 1: (1, 4, 7, 10, 13)}
# k-tiles (per parity) on the pool chain (A16 on DVE + mul on DVE + mul on
# GPSIMD).  Early-mid kt only so the chain's ~8-step latency stays ahead of
# the kt=15 stop batch.
POOLKT = ()
POOL_LAG = 8.5


def _register_dve_exp():
    """Register two custom DVE ops (cubic+2 squarings, then 3 squarings).
    TRN2 DVE = v3: 8 ALU stages per pass, so exp needs two chained ops.
    Coefficients are call-time scalars, so the same pass-A op serves both
    the z/32 and z/16 schemes."""
    if "dve_ops" in _CACHE:
        return _CACHE["dve_ops"]
    import concourse.dve_ops as dops
    from concourse.dve_ops import DveOp
    from concourse.dve_spec import Spec, Src0, C0, C1, C2, One, sq
    from concourse.dve_uop import DveOpSpec
    from concourse.dve_spec import lower, _has_src1 as has_src1
    import numpy as np_

    def _ref_expa(in0, in1, c0, c1, c2):
        f = np_.float32
        u = in0.astype(f)
        q = (f(1.0) + u * (f(c0) + u * (f(c1) + u * f(c2)))).astype(f)
        q = (q * q).astype(f)
        return (q * q).astype(f)

    def _ref_expb(in0, in1, c0, c1, c2):
        f = np_.float32
        q = (in0.astype(f) * in0.astype(f)).astype(f)
        q = (q * q).astype(f)
        return (q * q).astype(f)

    body_a = sq(sq(One + Src0 * (C0 + Src0 * (C1 + Src0 * C2))))
    body_b = sq(sq(sq(Src0)))
    spec_a = Spec(body=body_a, reference=_ref_expa)
    spec_b = Spec(body=body_b, reference=_ref_expb)

    ops = []
    for name, spec in (("EXP2A_MHA", spec_a), ("EXP2B_MHA", spec_b)):
        if name in dops._SUB_OPCODE_FOR_NAME:
            op = next(o for o in dops.OPS if o.name == name)
            ops.append(op)
            continue
        row = max(dops._SUB_OPCODE_FOR_NAME.values()) + 1
        assert row < 0x20
        dops._SUB_OPCODE_FOR_NAME[name] = row
        shas = {}
        for ver in ("v3", "v4"):
            try:
                spec_obj = DveOpSpec(name=name, opcode=row,
                                     uops=lower(spec, ver=ver),
                                     rd1_en=has_src1(spec))
                shas[ver] = spec_obj.sha(ver)
            except Exception:
                pass
        op = DveOp(name, spec, subdim=False, uops_sha=shas)
        dops.OPS.append(op)
        dops.CUSTOM_DVE_SPECS[name] = op.spec
        ops.append(op)
    _CACHE["dve_ops"] = ops
    return ops


def _build(reps=1):
    import concourse.tile as tile
    from concourse import bacc, mybir

    f32 = mybir.dt.float32
    f16 = mybir.dt.float16
    Exp = mybir.ActivationFunctionType.Exp

    nc = bacc.Bacc("TRN2", target_bir_lowering=False, debug=False,
                   num_devices=NCORES)
    qt_d = nc.dram_tensor("qt", [HPC, 128, 1024], f16,
                          kind="ExternalInput").ap()
    kt_d = nc.dram_tensor("kt", [HPC, 128, 1024], f16,
                          kind="ExternalInput").ap()
    kw_d = nc.dram_tensor("kw", [HPC, 128, 1024], f16,
                          kind="ExternalInput").ap()
    vt_d = nc.dram_tensor("vt", [HPC, 128, NT, 64], f16,
                          kind="ExternalInput").ap()
    o_d = nc.dram_tensor("o", [HPC, 2, 2, 128, 260], f32,
                         kind="ExternalOutput").ap()

    expa, expb = _register_dve_exp()

    with tile.TileContext(nc) as tc:
        with (
            tc.tile_pool(name="qsp", bufs=2) as qpool,
            tc.tile_pool(name="ksp", bufs=2) as kpool,
            tc.tile_pool(name="ksw", bufs=2) as wpool,
            tc.tile_pool(name="vap", bufs=3) as vpool,
            tc.tile_pool(name="pt", bufs=10) as ppool,
            tc.tile_pool(name="et", bufs=4) as epool,
            tc.tile_pool(name="pm", bufs=2) as mpool,
            tc.tile_pool(name="osb", bufs=3) as bpool,
            tc.tile_pool(name="spsum", bufs=3, space="PSUM") as spool,
            tc.tile_pool(name="opsum", bufs=1, space="PSUM") as opool,
        ):
            def emit_in_dmas(h, first=False):
                """Issue all input DMAs for head h; returns its tiles."""
                qsp = qpool.tile([128, 1024], f16, tag="qsp", name="qsp")
                ksp = kpool.tile([128, 1024], f16, tag="ksp", name="ksp")
                ksw = wpool.tile([128, 1024], f16, tag="ksw", name="ksw")
                vaug = vpool.tile([128, NT * 65], f16, tag="vaug", name="vaug")
                v3 = vaug.rearrange("p (t c) -> p t c", c=65)
                if first:
                    nc.sync.dma_start(ksp[:, 0:128], kt_d[h][:, 0:128])
                    nc.sync.dma_start(qsp[:, 0:512], qt_d[h][:, 0:512])
                    nc.scalar.dma_start(qsp[:, 512:1024],
                                        qt_d[h][:, 512:1024])
                    nc.scalar.dma_start(ksp[:, 128:512], kt_d[h][:, 128:512])
                    nc.gpsimd.dma_start(ksp[:, 512:1024],
                                        kt_d[h][:, 512:1024])
                    nc.gpsimd.dma_start(v3[:, :, 0:64], vt_d[h])
                    nc.gpsimd.dma_start(ksw, kw_d[h])
                else:
                    nc.sync.dma_start(qsp, qt_d[h])
                    nc.sync.dma_start(ksp, kt_d[h])
                    nc.sync.dma_start(v3[:, :, 0:64], vt_d[h])
                    nc.sync.dma_start(ksw, kw_d[h])
                nc.gpsimd.memset(v3[:, :, 64], 1.0)
                return {"qsp": qsp, "ksp": ksp, "ksw": ksw, "vaug": vaug}

            def lhs_k(t, qh, kt):
                """lhsT [64, 128] for k-tile kt at q-parity qh's range."""
                if kt < 8:   # even k tile
                    if qh == 0:
                        return t["ksp"][0:64, 128 * kt:128 * kt + 128]
                    return t["ksw"][64:128, 128 * kt:128 * kt + 128]
                kk = kt - 8
                if qh == 0:
                    return t["ksw"][0:64, 128 * kk:128 * kk + 128]
                return t["ksp"][64:128, 128 * kk:128 * kk + 128]

            def emit_s(t, st, sq):
                qh, kt = divmod(st, NT)
                sT = spool.tile([128, 1024], f32, tag="sT", name="sT")
                w = lhs_k(t, qh, kt)
                q0 = 64 * qh
                for c in range(2):
                    nc.tensor.matmul(
                        sT[:, 512 * c:512 * c + 512], w,
                        t["qsp"][q0:q0 + 64, 512 * c:512 * c + 512],
                        start=True, stop=True)
                pT = ppool.tile([128, 1024], f16, tag="pT", name="pT")
                if kt in OFFKT_P[qh]:
                    et = epool.tile([128, 1024], f16, tag="et", name="et")
                    nc.vector._custom_dve(expa, out=et, in0=sT,
                                          s0=EXP_C1, s1=EXP_C2, imm2=EXP_C3)
                    nc.vector._custom_dve(expb, out=pT, in0=et)
                elif kt in POOLKT:
                    et = epool.tile([128, 1024], f16, tag="et", name="et")
                    nc.vector._custom_dve(expa, out=et, in0=sT,
                                          s0=Z16_C1, s1=Z16_C2, imm2=Z16_C3)
                    m1 = mpool.tile([128, 1024], f16, tag="pm1", name="pm1")
                    nc.vector.tensor_mul(m1, et, et)
                    nc.gpsimd.tensor_mul(pT, m1, m1)
                else:
                    nc.scalar.activation(pT, sT, Exp, scale=ACT_SCALE)
                sq[st] = [pT, 2]

            def emit_pv(t, st, half, sq, octx, h):
                qh, kt = divmod(st, NT)
                key = ("o", half)
                if kt == 0:
                    octx[key] = opool.tile([128, 512], f32,
                                           tag=f"o{half}", name=f"o{half}")
                ob = octx[key]
                ent = sq[st]
                pT = ent[0]
                for j in range(4):
                    qt = 4 * half + j
                    # start=True zeroes the ENTIRE psum bank, so only the
                    # bank's very first matmul may set it; the other slots
                    # accumulate onto the start-cleared bank
                    nc.tensor.matmul(
                        ob[:, 65 * j:65 * j + 65],
                        pT[:, 128 * qt:128 * qt + 128],
                        t["vaug"][:, 65 * kt:65 * kt + 65],
                        start=(kt == 0 and j == 0), stop=(kt == NT - 1))
                ent[1] -= 1
                if ent[1] == 0:
                    del sq[st]

            def emit_drain(octx, h, qh, half, tail_mode=False):
                """Drain one O bank (psum->sbuf copy; half 0 on ACT a step
                after the last PV, half 1 on DVE in tail mode) and DMA the
                raw numerator+denominator block out; host normalizes."""
                osb = bpool.tile([128, 260], f32, tag=f"osb{half}",
                                 name=f"osb{half}")
                ob = octx.pop(("o", half))
                if half == 1 and tail_mode:
                    nc.vector.tensor_copy(osb, ob[:, 0:260])
                else:
                    nc.scalar.copy(osb, ob[:, 0:260])
                eng = nc.scalar if (tail_mode and half == 1) else nc.sync
                eng.dma_start(o_d[h][qh][half], osb)

            seq = [i % HPC for i in range(HPC * reps)]

            def handle_pv(gst, phi, pst, phalf):
                emit_pv(tiles[phi], pst, phalf, sqs[phi],
                        octxs.setdefault(phi, {}), seq[phi])
                if pst % NT == NT - 1:
                    if (phi, pst) in last_par1:
                        dq.append((gst, phi, pst // NT, phalf))
                    else:
                        # both drains on ACT, 1-2 steps late so the copy's
                        # PV-completion wait never stalls queued exp work
                        dq.append((gst + 1 + phalf, phi, pst // NT, phalf))
                if pst == NSTEP - 1 and phalf == 1:
                    # head fully retired; release tile refs
                    tiles.pop(phi - 1, None)

            tiles = {0: emit_in_dmas(seq[0], first=True)}

            # Warm the ACT exp table during the initial DMA fill.
            warm = bpool.tile([128, 1], f32, tag="warm", name="warm")
            nc.gpsimd.memset(warm, 0.0)
            warm_o = bpool.tile([128, 1], f16, tag="warmo", name="warmo")
            nc.scalar.activation(warm_o, warm, Exp, scale=1.0)
            pvq = []   # (due_gst, emit_order, hi, st, half)
            sqs = {}   # hi -> {st: [pT, refcount]}
            octxs = {}  # hi -> {("o", half): tile}
            order = 0
            total = len(seq) * NSTEP
            max_due = [0, 0]
            dq = []
            last_par1 = {(len(seq) - 1, st) for st in range(NT, NSTEP)}
            gst = 0
            while gst < total + 12:
                # overdue PVs (fractional dues, e.g. the kt=15 batch) must
                # land BEFORE this step's emit_s
                while pvq and pvq[0][0] <= gst - 0.5:
                    _, _, phi, pst, phalf = pvq.pop(0)
                    handle_pv(gst, phi, pst, phalf)
                while dq and dq[0][0] <= gst:
                    _, phi, pqh, phalf = dq.pop(0)
                    emit_drain(octxs[phi], seq[phi], pqh, phalf)
                if gst < total:
                    hi, st = divmod(gst, NSTEP)
                    qh, kt = divmod(st, NT)
                    h = seq[hi]
                    sq = sqs.setdefault(hi, {})
                    emit_s(tiles[hi], st, sq)
                    if kt == NT - 1:
                        lag_a = lag_b = 1.5
                    elif kt == 0:
                        lag_a, lag_b = 4, 5  # O-bank drain window
                    elif kt in POOLKT:
                        lag_a = lag_b = POOL_LAG
                    elif kt in OFFKT_P[qh]:
                        lag_a = lag_b = 4    # 2-op DVE exp latency
                    else:
                        lag_a = lag_b = 2
                    # clamp: PV emission per half follows kt order (the
                    # kt==0 start=True matmul zeroes the whole bank and the
                    # kt==15 stop batch must be last before the drain)
                    due_a = max(gst + lag_a, max_due[0])
                    due_b = max(gst + lag_b, max_due[1])
                    max_due[0], max_due[1] = due_a, due_b
                    pvq.append((due_a, order, hi, st, 0)); order += 1
                    pvq.append((due_b, order, hi, st, 1)); order += 1
                    pvq.sort()
                    if st == 2 and hi + 1 < len(seq):
                        tiles[hi + 1] = emit_in_dmas(seq[hi + 1])
                while pvq and pvq[0][0] <= gst:
                    _, _, phi, pst, phalf = pvq.pop(0)
                    handle_pv(gst, phi, pst, phalf)
                if gst >= total:
                    while dq:
                        _, phi, pqh, phalf = dq.pop(0)
                        emit_drain(octxs[phi], seq[phi], pqh, phalf,
                                   tail_mode=True)
                gst += 1

    nc.compile()
    return nc


def get_nc(reps=1):
    key = f"nc{reps}"
    if key not in _CACHE:
        _CACHE[key] = _build(reps)
    return _CACHE[key]


def _pack_inputs(q, k, v):
    """Host-side cast + layout packing (see module docstring)."""
    qf = (np.asarray(q, dtype=np.float32) * np.float32(2.0 ** -4)) \
        .astype(np.float16).reshape(HEADS, N, D)
    kf = (np.asarray(k, dtype=np.float32) * np.float32(2.0 ** -4)) \
        .astype(np.float16).reshape(HEADS, N, D)
    vf = np.asarray(v, dtype=np.float32).astype(np.float16) \
        .reshape(HEADS, N, D)
    # [h, n, d] -> [h, 128, 1024]: partition = (n&1)*64 + d, col = n//2
    qt = qf.reshape(HEADS, 1024, 2, 64).transpose(0, 2, 3, 1) \
        .reshape(HEADS, 128, 1024)
    kt = kf.reshape(HEADS, 1024, 2, 64).transpose(0, 2, 3, 1) \
        .reshape(HEADS, 128, 1024)
    kw = kt.reshape(HEADS, 2, 64, 1024)[:, ::-1].reshape(HEADS, 128, 1024)
    # v rows n = t*256 + p*2 + two -> vt[h, p, two*8 + t, d]
    vt = vf.reshape(HEADS, 8, 128, 2, 64).transpose(0, 2, 3, 1, 4) \
        .reshape(HEADS, 128, NT, 64)
    c = np.ascontiguousarray
    return [
        {"qt": c(qt[i * HPC:(i + 1) * HPC]),
         "kt": c(kt[i * HPC:(i + 1) * HPC]),
         "kw": c(kw[i * HPC:(i + 1) * HPC]),
         "vt": c(vt[i * HPC:(i + 1) * HPC])}
        for i in range(NCORES)
    ]


def _unpack_output(raws):
    """raws: NCORES x [HPC, 2, 2, 128, 260] f32 raw numerator+denominator.
    Host divide + relayout to [B, H, N, D]."""
    raw = np.concatenate(raws, axis=0)  # [HEADS, 2, 2, 128, 260]
    rr = raw.reshape(HEADS, 2, 2, 128, 4, 65)
    num = rr[..., 0:64]
    den = rr[..., 64:65]
    res = num / den  # [h, qh, half, p, j, d]
    # n = (half*4 + j)*256 + p*2 + qh
    out = res.transpose(0, 2, 4, 3, 1, 5).reshape(HEADS, N, D)
    return np.ascontiguousarray(out.reshape(B, H, N, D).astype(np.float32))


def kernel(q, k, v):
    from concourse.bass_utils import run_bass_kernel_spmd

    nc = get_nc()
    in_maps = _pack_inputs(q, k, v)
    res = run_bass_kernel_spmd(nc, in_maps, list(range(NCORES)))
    return _unpack_output([res.results[c]["o"] for c in range(NCORES)])
